# revision 6
# baseline (speedup 1.0000x reference)
"""DeepSeek MoE layer on 8 Trainium2 NeuronCores.

Strategy: data-parallel over tokens (N = B*T = 8192 -> 1024 tokens/core).
Every core holds the full weight set and processes its token shard through
the router, both shared SwiGLU experts and (densely, gate-masked) all 8
routed GELU experts.  All large matmuls run in float32r (full fp32 data,
fast PE mode, ~1.5e-4 rel err); the router runs in plain fp32 so top-2
expert selection matches the fp32 reference exactly.

Layout is feature-major throughout: activations live as [feature, token]
tiles so matmul outputs chain into the next matmul's moving operand with
no transposes.  Host preprocessing is limited to transposing weights /
slicing tokens (cached across calls); all FLOPs happen on device.
"""

import numpy as np

import concourse.bass as bass
import concourse.mybir as mybir
from concourse.tile import TileContext
from concourse.bass_utils import run_bass_kernel_spmd

# ---------------------------------------------------------------------------
# problem constants (hardcoded per harness contract)
D = 1024          # d_model
HS = 2048         # shared expert hidden
HR = 1024         # routed expert hidden
E = 8             # routed experts
NS = 2            # shared experts
TOPK = 2
B, T = 4, 2048
N_CORES = 8
TOK = (B * T) // N_CORES      # tokens per core
P = 128
NKD = D // P                  # 8 k-tiles over d_model
NKH_S = HS // P               # 16 k-tiles over shared hidden
NKH_R = HR // P               # 8 k-tiles over routed hidden
NV = 512                      # moving-dim tile (tokens per matmul)
NN = TOK // NV                # 2 token column tiles

F32 = mybir.dt.float32
F32R = mybir.dt.float32r


def _legalize_waits(nc):
    """Split multi-wait instructions into single-wait NOP prefixes.

    The walrus pass list used by the bass2jax compile path has no sync
    legalization pass and cayman 64B instructions carry exactly one wait
    slot, so any instruction with >1 sem-waits fails codegen.  Rewrite
    every such instruction into wait-only same-engine NOPs followed by
    the instruction carrying the final wait; semantics are identical.
    """
    n_split = 0
    for fn in nc.m.functions:
        for blk in fn.blocks:
            out = []
            changed = False
            for inst in blk.instructions:
                si = inst.sync_info
                waits = list(si.on_wait) if si is not None and si.on_wait else []
                if len(waits) > 1:
                    for w in waits[:-1]:
                        nop = mybir.InstNoOp(
                            name=nc.get_next_instruction_name(),
                            engine=inst.engine,
                            bass_nofuse=True,
                            sync_info=mybir.SyncInfo(on_wait=[w], on_update=[]),
                        )
                        nc.register_instruction(nop)
                        out.append(nop)
                    si.on_wait = [waits[-1]]
                    inst.sync_info = si
                    n_split += 1
                    changed = True
                out.append(inst)
            if changed:
                blk.instructions = out
    return n_split


def _build_nc():
    nc = bass.Bass()

    xT = nc.declare_dram_parameter("xT", [D, TOK], F32R, isOutput=False)
    tC = nc.declare_dram_parameter("tC", [P, NKD], F32, isOutput=False)
    wrT = nc.declare_dram_parameter("wrT", [D, 2 * E], F32, isOutput=False)
    iota = nc.declare_dram_parameter("iota", [P, E], F32, isOutput=False)
    ident = nc.declare_dram_parameter("ident", [P, P], F32, isOutput=False)
    sw1 = nc.declare_dram_parameter("sw1", [NS, D, HS], F32R, isOutput=False)
    sw3 = nc.declare_dram_parameter("sw3", [NS, D, HS], F32R, isOutput=False)
    sw2 = nc.declare_dram_parameter("sw2", [NS, HS, D], F32R, isOutput=False)
    rw1 = nc.declare_dram_parameter("rw1", [E, D, HR], F32R, isOutput=False)
    rw2 = nc.declare_dram_parameter("rw2", [E, HR, D], F32R, isOutput=False)
    yout = nc.declare_dram_parameter("yout", [D, TOK], F32, isOutput=True)

    AF = mybir.ActivationFunctionType
    ALU = mybir.AluOpType
    AX = mybir.AxisListType

    with TileContext(nc) as tc:
        with (
            tc.tile_pool(name="xpool", bufs=1) as xpool,
            tc.tile_pool(name="cpool", bufs=1) as cpool,
            tc.tile_pool(name="hpool", bufs=1) as hpool,
            tc.tile_pool(name="ypool", bufs=1) as ypool,
            tc.tile_pool(name="wpool", bufs=2) as wpool,
            tc.tile_pool(name="spool", bufs=3) as spool,
            tc.tile_pool(name="gpool", bufs=1) as gpool,
            tc.tile_pool(name="pp_h1", bufs=2, space="PSUM") as pp_h1,
            tc.tile_pool(name="pp_h3", bufs=2, space="PSUM") as pp_h3,
            tc.tile_pool(name="pp_y", bufs=2, space="PSUM") as pp_y,
            tc.tile_pool(name="pp_s", bufs=2, space="PSUM") as pp_s,
        ):
            # ---------------- preload ----------------
            x_t = xpool.tile([P, NKD * TOK], F32R)       # [p, kd*TOK + tok]
            nc.sync.dma_start(
                out=x_t[:].rearrange("p (kd t) -> p kd t", t=TOK),
                in_=xT.rearrange("(kd p) t -> p kd t", p=P),
            )
            id_t = cpool.tile([P, P], F32)
            nc.sync.dma_start(out=id_t[:], in_=ident[:, :])
            io_t = cpool.tile([P, E], F32)
            nc.sync.dma_start(out=io_t[:], in_=iota[:, :])
            tc_t = cpool.tile([P, NKD], F32)
            nc.sync.dma_start(out=tc_t[:], in_=tC[:, :])
            wr_t = cpool.tile([P, NKD * 2 * E], F32)     # [p, kd*16 + col]
            nc.sync.dma_start(
                out=wr_t[:].rearrange("p (kd c) -> p kd c", c=2 * E),
                in_=wrT.rearrange("(kd p) c -> p kd c", p=P),
            )
            ones32 = cpool.tile([1, P], F32)
            nc.vector.memset(ones32[:], 1.0)
            ones_r = cpool.tile([1, P], F32R)
            nc.vector.tensor_copy(ones_r[:], ones32[:])

            # ---------------- router (fp32) ----------------
            # t-embedding logit contribution: tl[1, E]
            tl_ps = pp_s.tile([1, E], F32, space="PSUM", tag="ps_small")
            for kd in range(NKD):
                nc.tensor.matmul(
                    tl_ps[:],
                    tc_t[:, kd:kd + 1],
                    wr_t[:, kd * 2 * E + E:(kd + 1) * 2 * E],
                    start=(kd == 0), stop=(kd == NKD - 1),
                )
            tl_s = spool.tile([1, E], F32, tag="tl")
            nc.vector.tensor_copy(tl_s[:], tl_ps[:])

            gates_T = gpool.tile([E, TOK], F32R)
            for tt in range(TOK // P):
                L_ps = pp_s.tile([P, E], F32, space="PSUM", tag="ps_small")
                for kd in range(NKD):
                    nc.tensor.matmul(
                        L_ps[:],
                        x_t[:, kd * TOK + tt * P: kd * TOK + (tt + 1) * P].bitcast(F32),
                        wr_t[:, kd * 2 * E:kd * 2 * E + E],
                        start=(kd == 0), stop=False,
                    )
                nc.tensor.matmul(L_ps[:], ones32[0:1, :], tl_s[0:1, :],
                                 start=False, stop=True)

                Lt = spool.tile([P, E], F32, tag="rt_L")
                nc.vector.tensor_copy(Lt[:], L_ps[:])
                St = spool.tile([P, E], F32, tag="rt_S")
                nc.scalar.activation(St[:], Lt[:], AF.Sigmoid)

                # top-1 (lowest index wins ties, matching jax.lax.top_k)
                m1 = spool.tile([P, 1], F32, tag="rt_m1")
                nc.vector.reduce_max(m1[:], Lt[:], axis=AX.X)
                eq1 = spool.tile([P, E], F32, tag="rt_eq1")
                nc.vector.tensor_scalar(eq1[:], Lt[:], m1[:, 0:1], None, op0=ALU.is_ge)
                pen1 = spool.tile([P, E], F32, tag="rt_pen1")
                nc.vector.tensor_scalar(pen1[:], eq1[:], -1e9, 1e9,
                                        op0=ALU.mult, op1=ALU.add)
                ix1 = spool.tile([P, E], F32, tag="rt_ix1")
                nc.vector.tensor_tensor(out=ix1[:], in0=io_t[:], in1=pen1[:], op=ALU.add)
                i1 = spool.tile([P, 1], F32, tag="rt_i1")
                nc.vector.tensor_reduce(i1[:], ix1[:], axis=AX.X, op=ALU.min)
                mask1 = spool.tile([P, E], F32, tag="rt_mask1")
                nc.vector.tensor_scalar(mask1[:], io_t[:], i1[:, 0:1], None,
                                        op0=ALU.is_equal)

                # top-2
                neg1 = spool.tile([P, E], F32, tag="rt_neg1")
                nc.vector.tensor_scalar(neg1[:], mask1[:], -1e30, None, op0=ALU.mult)
                L2 = spool.tile([P, E], F32, tag="rt_L2")
                nc.vector.tensor_tensor(out=L2[:], in0=Lt[:], in1=neg1[:], op=ALU.add)
                m2 = spool.tile([P, 1], F32, tag="rt_m2")
                nc.vector.reduce_max(m2[:], L2[:], axis=AX.X)
                eq2 = spool.tile([P, E], F32, tag="rt_eq2")
                nc.vector.tensor_scalar(eq2[:], L2[:], m2[:, 0:1], None, op0=ALU.is_ge)
                pen2 = spool.tile([P, E], F32, tag="rt_pen2")
                nc.vector.tensor_scalar(pen2[:], eq2[:], -1e9, 1e9,
                                        op0=ALU.mult, op1=ALU.add)
                ix2 = spool.tile([P, E], F32, tag="rt_ix2")
                nc.vector.tensor_tensor(out=ix2[:], in0=io_t[:], in1=pen2[:], op=ALU.add)
                i2 = spool.tile([P, 1], F32, tag="rt_i2")
                nc.vector.tensor_reduce(i2[:], ix2[:], axis=AX.X, op=ALU.min)
                mask2 = spool.tile([P, E], F32, tag="rt_mask2")
                nc.vector.tensor_scalar(mask2[:], io_t[:], i2[:, 0:1], None,
                                        op0=ALU.is_equal)

                mask = spool.tile([P, E], F32, tag="rt_mask")
                nc.vector.tensor_tensor(out=mask[:], in0=mask1[:], in1=mask2[:], op=ALU.add)
                sm = spool.tile([P, E], F32, tag="rt_sm")
                nc.vector.tensor_tensor(out=sm[:], in0=St[:], in1=mask[:], op=ALU.mult)
                den = spool.tile([P, 1], F32, tag="rt_den")
                nc.vector.reduce_sum(den[:], sm[:], axis=AX.X)
                den2 = spool.tile([P, 1], F32, tag="rt_den2")
                nc.vector.tensor_scalar(den2[:], den[:], 1e-9, None, op0=ALU.add)
                rec = spool.tile([P, 1], F32, tag="rt_rec")
                nc.vector.reciprocal(rec[:], den2[:])
                rec4 = spool.tile([P, 1], F32, tag="rt_rec4")
                nc.vector.tensor_scalar(rec4[:], rec[:], 0.25, None, op0=ALU.mult)
                gates = spool.tile([P, E], F32, tag="rt_gates")
                nc.vector.tensor_scalar(gates[:], sm[:], rec4[:, 0:1], None, op0=ALU.mult)

                tr_ps = pp_s.tile([E, P], F32, space="PSUM", tag="ps_small")
                nc.tensor.transpose(out=tr_ps[:], in_=gates[:], identity=id_t[:])
                nc.vector.tensor_copy(gates_T[:, tt * P:(tt + 1) * P], tr_ps[:])

            # ---------------- experts ----------------
            h_t = hpool.tile([P, NKH_S * TOK], F32R)     # [p, kh*TOK + tok]
            y_t = ypool.tile([P, NKD * TOK], F32)        # [p, m2*TOK + tok]

            # --- shared experts (SwiGLU, w3 pre-scaled by 1/8 on host) ---
            for e in range(NS):
                w3blks = {}
                for m in range(NKH_S):
                    w1blk = wpool.tile([P, NKD * P], F32R, tag="w_in")
                    nc.sync.dma_start(
                        out=w1blk[:].rearrange("p (kd m) -> p kd m", m=P),
                        in_=sw1[e, :, m * P:(m + 1) * P].rearrange("(kd p) m -> p kd m", p=P),
                    )
                    w3blk = wpool.tile([P, NKD * P], F32R, tag="w_in3")
                    nc.sync.dma_start(
                        out=w3blk[:].rearrange("p (kd m) -> p kd m", m=P),
                        in_=sw3[e, :, m * P:(m + 1) * P].rearrange("(kd p) m -> p kd m", p=P),
                    )
                    for n in range(NN):
                        ph1 = pp_h1.tile([P, NV], F32, space="PSUM", tag="ph1")
                        ph3 = pp_h3.tile([P, NV], F32, space="PSUM", tag="ph3")
                        for kd in range(NKD):
                            xs = x_t[:, kd * TOK + n * NV: kd * TOK + n * NV + NV]
                            nc.tensor.matmul(ph1[:], w1blk[:, kd * P:(kd + 1) * P], xs,
                                             start=(kd == 0), stop=(kd == NKD - 1))
                        for kd in range(NKD):
                            xs = x_t[:, kd * TOK + n * NV: kd * TOK + n * NV + NV]
                            nc.tensor.matmul(ph3[:], w3blk[:, kd * P:(kd + 1) * P], xs,
                                             start=(kd == 0), stop=(kd == NKD - 1))
                        sil = spool.tile([P, NV], F32, tag="sil")
                        nc.scalar.activation(sil[:], ph1[:], AF.Silu)
                        hs = h_t[:, m * TOK + n * NV: m * TOK + n * NV + NV]
                        nc.vector.tensor_tensor(out=hs, in0=sil[:], in1=ph3[:], op=ALU.mult)

                for m2 in range(NKD):
                    w2blk = wpool.tile([P, NKH_S * P], F32R, tag="w_out")
                    nc.sync.dma_start(
                        out=w2blk[:].rearrange("p (kh m) -> p kh m", m=P),
                        in_=sw2[e, :, m2 * P:(m2 + 1) * P].rearrange("(kh p) m -> p kh m", p=P),
                    )
                    for n in range(NN):
                        py = pp_y.tile([P, NV], F32, space="PSUM", tag="py")
                        for kh in range(NKH_S):
                            hsl = h_t[:, kh * TOK + n * NV: kh * TOK + n * NV + NV]
                            nc.tensor.matmul(py[:], w2blk[:, kh * P:(kh + 1) * P], hsl,
                                             start=(kh == 0), stop=(kh == NKH_S - 1))
                        ysl = y_t[:, m2 * TOK + n * NV: m2 * TOK + n * NV + NV]
                        if e == 0:
                            nc.vector.tensor_copy(ysl, py[:])
                        else:
                            nc.vector.tensor_tensor(out=ysl, in0=ysl, in1=py[:], op=ALU.add)

            # --- routed experts (GELU MLP, dense with gate masking) ---
            for e in range(E):
                gb_row = spool.tile([1, TOK], F32R, tag="gbrow")
                nc.sync.dma_start(out=gb_row[:], in_=gates_T[e:e + 1, :])
                gb = spool.tile([P, TOK], F32, tag="gb")
                for n in range(NN):
                    gb_ps = pp_s.tile([P, NV], F32, space="PSUM", tag="ps_small")
                    nc.tensor.matmul(gb_ps[:], ones_r[0:1, :],
                                     gb_row[0:1, n * NV:(n + 1) * NV],
                                     start=True, stop=True)
                    nc.vector.tensor_copy(gb[:, n * NV:(n + 1) * NV], gb_ps[:])

                for m in range(NKH_R):
                    w1blk = wpool.tile([P, NKD * P], F32R, tag="w_in")
                    nc.sync.dma_start(
                        out=w1blk[:].rearrange("p (kd m) -> p kd m", m=P),
                        in_=rw1[e, :, m * P:(m + 1) * P].rearrange("(kd p) m -> p kd m", p=P),
                    )
                    for n in range(NN):
                        ph = pp_h1.tile([P, NV], F32, space="PSUM", tag="ph1")
                        for kd in range(NKD):
                            xs = x_t[:, kd * TOK + n * NV: kd * TOK + n * NV + NV]
                            nc.tensor.matmul(ph[:], w1blk[:, kd * P:(kd + 1) * P], xs,
                                             start=(kd == 0), stop=(kd == NKD - 1))
                        hg = spool.tile([P, NV], F32, tag="hg")
                        nc.scalar.activation(hg[:], ph[:], AF.Gelu)
                        hs = h_t[:, m * TOK + n * NV: m * TOK + n * NV + NV]
                        nc.vector.tensor_tensor(out=hs, in0=hg[:],
                                                in1=gb[:, n * NV:(n + 1) * NV], op=ALU.mult)

                for m2 in range(NKD):
                    w2blk = wpool.tile([P, NKH_S * P], F32R, tag="w_out")
                    nc.sync.dma_start(
                        out=w2blk[:, :NKH_R * P].rearrange("p (kh m) -> p kh m", m=P),
                        in_=rw2[e, :, m2 * P:(m2 + 1) * P].rearrange("(kh p) m -> p kh m", p=P),
                    )
                    for n in range(NN):
                        py = pp_y.tile([P, NV], F32, space="PSUM", tag="py")
                        for kh in range(NKH_R):
                            hsl = h_t[:, kh * TOK + n * NV: kh * TOK + n * NV + NV]
                            nc.tensor.matmul(py[:], w2blk[:, kh * P:(kh + 1) * P], hsl,
                                             start=(kh == 0), stop=(kh == NKH_R - 1))
                        ysl = y_t[:, m2 * TOK + n * NV: m2 * TOK + n * NV + NV]
                        nc.vector.tensor_tensor(out=ysl, in0=ysl, in1=py[:], op=ALU.add)

            # ---------------- store ----------------
            nc.sync.dma_start(
                out=yout.rearrange("(m2 p) t -> p m2 t", p=P),
                in_=y_t[:].rearrange("p (m2 t) -> p m2 t", t=TOK),
            )

    _legalize_waits(nc)
    return nc


_CACHE = {}


def _prep_weights(t_emb, W_router, router_bias, s_w1, s_w3, s_w2, r_w1, r_w2):
    key = tuple(id(a) for a in (t_emb, W_router, router_bias, s_w1, s_w3, s_w2, r_w1, r_w2))
    hit = _CACHE.get("wkey")
    if hit is not None and hit[0] == key:
        return hit[1]
    assert np.all(np.asarray(router_bias) == 0.0), "kernel assumes zero router bias"
    c = np.ascontiguousarray
    f = np.float32
    prep = dict(
        wrT=c(np.asarray(W_router, f).T),                      # [2D, E] -> split cols
        sw1=c(np.asarray(s_w1, f).transpose(0, 2, 1)),          # [NS, D, HS]
        sw3=c(np.asarray(s_w3, f).transpose(0, 2, 1) / 8.0),    # scale folds /(NS*(NS+TOPK))
        sw2=c(np.asarray(s_w2, f).transpose(0, 2, 1)),          # [NS, HS, D]
        rw1=c(np.asarray(r_w1, f).transpose(0, 2, 1)),          # [E, D, HR]
        rw2=c(np.asarray(r_w2, f).transpose(0, 2, 1)),          # [E, HR, D]
        t_cols=[c(np.asarray(t_emb, f)[b].reshape(NKD, P).T) for b in range(B)],
        iota=c(np.broadcast_to(np.arange(E, dtype=f), (P, E))),
        ident=np.eye(P, dtype=f),
    )
    prep["wrT_packed"] = c(np.concatenate(
        [prep["wrT"][:D, :], prep["wrT"][D:, :]], axis=1))      # [D, 16]: x-part | t-part
    out = prep
    _CACHE["wkey"] = (key, out)
    return out


def kernel(x, t_emb, W_router, router_bias, s_w1, s_w3, s_w2, r_w1, r_w2):
    x = np.asarray(x, np.float32)
    pw = _prep_weights(t_emb, W_router, router_bias, s_w1, s_w3, s_w2, r_w1, r_w2)

    if "nc" not in _CACHE:
        _CACHE["nc"] = _build_nc()
    nc = _CACHE["nc"]

    xT_full = np.ascontiguousarray(x.reshape(B * T, D).T)       # [D, N]
    in_maps = []
    for cix in range(N_CORES):
        in_maps.append(dict(
            xT=np.ascontiguousarray(xT_full[:, cix * TOK:(cix + 1) * TOK]),
            tC=pw["t_cols"][cix * TOK // T],
            wrT=pw["wrT_packed"],
            iota=pw["iota"],
            ident=pw["ident"],
            sw1=pw["sw1"], sw3=pw["sw3"], sw2=pw["sw2"],
            rw1=pw["rw1"], rw2=pw["rw2"],
        ))

    res = run_bass_kernel_spmd(nc, in_maps, list(range(N_CORES)))

    out = np.empty((D, B * T), dtype=np.float32)
    for cix in range(N_CORES):
        out[:, cix * TOK:(cix + 1) * TOK] = res.results[cix]["yout"]
    return np.ascontiguousarray(out.T).reshape(B, T, D)


# revision 8
# speedup vs baseline: 1.0140x; 1.0140x over previous
"""DeepSeek MoE layer on 8 Trainium2 NeuronCores.

Strategy: data-parallel over tokens (N = B*T = 8192 -> 1024 tokens/core).
Every core holds the full weight set and processes its token shard through
the router, both shared SwiGLU experts and (densely, gate-masked) all 8
routed GELU experts.  All large matmuls run in float32r (full fp32 data,
fast PE mode, ~1.5e-4 rel err); the router runs in plain fp32 so top-2
expert selection matches the fp32 reference exactly.

Layout is feature-major throughout: activations live as [feature, token]
tiles so matmul outputs chain into the next matmul's moving operand with
no transposes.  Host preprocessing is limited to transposing weights /
slicing tokens (cached across calls); all FLOPs happen on device.
"""

import numpy as np

import concourse.bass as bass
import concourse.mybir as mybir
from concourse.tile import TileContext
from concourse.bass_utils import run_bass_kernel_spmd

# ---------------------------------------------------------------------------
# problem constants (hardcoded per harness contract)
D = 1024          # d_model
HS = 2048         # shared expert hidden
HR = 1024         # routed expert hidden
E = 8             # routed experts
NS = 2            # shared experts
TOPK = 2
B, T = 4, 2048
N_CORES = 8
TOK = (B * T) // N_CORES      # tokens per core
P = 128
NKD = D // P                  # 8 k-tiles over d_model
NKH_S = HS // P               # 16 k-tiles over shared hidden
NKH_R = HR // P               # 8 k-tiles over routed hidden
NV = 512                      # moving-dim tile (tokens per matmul)
NN = TOK // NV                # 2 token column tiles

F32 = mybir.dt.float32
F32R = mybir.dt.float32r


def _legalize_waits(nc):
    """Split multi-wait instructions into single-wait NOP prefixes.

    The walrus pass list used by the bass2jax compile path has no sync
    legalization pass and cayman 64B instructions carry exactly one wait
    slot, so any instruction with >1 sem-waits fails codegen.  Rewrite
    every such instruction into wait-only same-engine NOPs followed by
    the instruction carrying the final wait; semantics are identical.
    """
    n_split = 0
    for fn in nc.m.functions:
        for blk in fn.blocks:
            out = []
            changed = False
            for inst in blk.instructions:
                si = inst.sync_info
                waits = list(si.on_wait) if si is not None and si.on_wait else []
                if len(waits) > 1:
                    for w in waits[:-1]:
                        nop = mybir.InstNoOp(
                            name=nc.get_next_instruction_name(),
                            engine=inst.engine,
                            bass_nofuse=True,
                            sync_info=mybir.SyncInfo(on_wait=[w], on_update=[]),
                        )
                        nc.register_instruction(nop)
                        out.append(nop)
                    si.on_wait = [waits[-1]]
                    inst.sync_info = si
                    n_split += 1
                    changed = True
                out.append(inst)
            if changed:
                blk.instructions = out
    return n_split


def _build_nc():
    nc = bass.Bass()

    xT = nc.declare_dram_parameter("xT", [D, TOK], F32R, isOutput=False)
    tC = nc.declare_dram_parameter("tC", [P, NKD], F32, isOutput=False)
    wrT = nc.declare_dram_parameter("wrT", [D, 2 * E], F32, isOutput=False)
    iota = nc.declare_dram_parameter("iota", [P, E], F32, isOutput=False)
    ident = nc.declare_dram_parameter("ident", [P, P], F32, isOutput=False)
    sw1 = nc.declare_dram_parameter("sw1", [NS, D, HS], F32R, isOutput=False)
    sw3 = nc.declare_dram_parameter("sw3", [NS, D, HS], F32R, isOutput=False)
    sw2 = nc.declare_dram_parameter("sw2", [NS, HS, D], F32R, isOutput=False)
    rw1 = nc.declare_dram_parameter("rw1", [E, D, HR], F32R, isOutput=False)
    rw2 = nc.declare_dram_parameter("rw2", [E, HR, D], F32R, isOutput=False)
    yout = nc.declare_dram_parameter("yout", [D, TOK], F32, isOutput=True)

    AF = mybir.ActivationFunctionType
    ALU = mybir.AluOpType
    AX = mybir.AxisListType

    with TileContext(nc) as tc:
        with (
            tc.tile_pool(name="xpool", bufs=1) as xpool,
            tc.tile_pool(name="cpool", bufs=1) as cpool,
            tc.tile_pool(name="hpool", bufs=1) as hpool,
            tc.tile_pool(name="ypool", bufs=1) as ypool,
            tc.tile_pool(name="wpool", bufs=2) as wpool,
            tc.tile_pool(name="spool", bufs=3) as spool,
            tc.tile_pool(name="gpool", bufs=1) as gpool,
            tc.tile_pool(name="pp_h1", bufs=2, space="PSUM") as pp_h1,
            tc.tile_pool(name="pp_h3", bufs=2, space="PSUM") as pp_h3,
            tc.tile_pool(name="pp_y", bufs=2, space="PSUM") as pp_y,
            tc.tile_pool(name="pp_s", bufs=2, space="PSUM") as pp_s,
        ):
            # ---------------- preload ----------------
            x_t = xpool.tile([P, NKD * TOK], F32R)       # [p, kd*TOK + tok]
            for kd in range(NKD):
                nc.sync.dma_start(
                    out=x_t[:, kd * TOK:(kd + 1) * TOK],
                    in_=xT[kd * P:(kd + 1) * P, :],
                )
            id_t = cpool.tile([P, P], F32)
            nc.sync.dma_start(out=id_t[:], in_=ident[:, :])
            io_t = cpool.tile([P, E], F32)
            nc.sync.dma_start(out=io_t[:], in_=iota[:, :])
            tc_t = cpool.tile([P, NKD], F32)
            nc.sync.dma_start(out=tc_t[:], in_=tC[:, :])
            wr_t = cpool.tile([P, NKD * 2 * E], F32)     # [p, kd*16 + col]
            nc.sync.dma_start(
                out=wr_t[:].rearrange("p (kd c) -> p kd c", c=2 * E),
                in_=wrT.rearrange("(kd p) c -> p kd c", p=P),
            )
            ones32 = cpool.tile([1, P], F32)
            nc.vector.memset(ones32[:], 1.0)
            ones_r = cpool.tile([1, P], F32R)
            nc.vector.tensor_copy(ones_r[:], ones32[:])

            # ---------------- router (fp32) ----------------
            # t-embedding logit contribution: tl[1, E]
            tl_ps = pp_s.tile([1, E], F32, space="PSUM", tag="ps_small")
            for kd in range(NKD):
                nc.tensor.matmul(
                    tl_ps[:],
                    tc_t[:, kd:kd + 1],
                    wr_t[:, kd * 2 * E + E:(kd + 1) * 2 * E],
                    start=(kd == 0), stop=(kd == NKD - 1),
                )
            tl_s = spool.tile([1, E], F32, tag="tl")
            nc.vector.tensor_copy(tl_s[:], tl_ps[:])

            gates_T = gpool.tile([E, TOK], F32R)
            for tt in range(TOK // P):
                L_ps = pp_s.tile([P, E], F32, space="PSUM", tag="ps_small")
                for kd in range(NKD):
                    nc.tensor.matmul(
                        L_ps[:],
                        x_t[:, kd * TOK + tt * P: kd * TOK + (tt + 1) * P].bitcast(F32),
                        wr_t[:, kd * 2 * E:kd * 2 * E + E],
                        start=(kd == 0), stop=False,
                    )
                nc.tensor.matmul(L_ps[:], ones32[0:1, :], tl_s[0:1, :],
                                 start=False, stop=True)

                Lt = spool.tile([P, E], F32, tag="rt_L")
                nc.vector.tensor_copy(Lt[:], L_ps[:])
                St = spool.tile([P, E], F32, tag="rt_S")
                nc.scalar.activation(St[:], Lt[:], AF.Sigmoid)

                # top-1 (lowest index wins ties, matching jax.lax.top_k)
                m1 = spool.tile([P, 1], F32, tag="rt_m1")
                nc.vector.reduce_max(m1[:], Lt[:], axis=AX.X)
                eq1 = spool.tile([P, E], F32, tag="rt_eq1")
                nc.vector.tensor_scalar(eq1[:], Lt[:], m1[:, 0:1], None, op0=ALU.is_ge)
                pen1 = spool.tile([P, E], F32, tag="rt_pen1")
                nc.vector.tensor_scalar(pen1[:], eq1[:], -1e9, 1e9,
                                        op0=ALU.mult, op1=ALU.add)
                ix1 = spool.tile([P, E], F32, tag="rt_ix1")
                nc.vector.tensor_tensor(out=ix1[:], in0=io_t[:], in1=pen1[:], op=ALU.add)
                i1 = spool.tile([P, 1], F32, tag="rt_i1")
                nc.vector.tensor_reduce(i1[:], ix1[:], axis=AX.X, op=ALU.min)
                mask1 = spool.tile([P, E], F32, tag="rt_mask1")
                nc.vector.tensor_scalar(mask1[:], io_t[:], i1[:, 0:1], None,
                                        op0=ALU.is_equal)

                # top-2
                neg1 = spool.tile([P, E], F32, tag="rt_neg1")
                nc.vector.tensor_scalar(neg1[:], mask1[:], -1e30, None, op0=ALU.mult)
                L2 = spool.tile([P, E], F32, tag="rt_L2")
                nc.vector.tensor_tensor(out=L2[:], in0=Lt[:], in1=neg1[:], op=ALU.add)
                m2 = spool.tile([P, 1], F32, tag="rt_m2")
                nc.vector.reduce_max(m2[:], L2[:], axis=AX.X)
                eq2 = spool.tile([P, E], F32, tag="rt_eq2")
                nc.vector.tensor_scalar(eq2[:], L2[:], m2[:, 0:1], None, op0=ALU.is_ge)
                pen2 = spool.tile([P, E], F32, tag="rt_pen2")
                nc.vector.tensor_scalar(pen2[:], eq2[:], -1e9, 1e9,
                                        op0=ALU.mult, op1=ALU.add)
                ix2 = spool.tile([P, E], F32, tag="rt_ix2")
                nc.vector.tensor_tensor(out=ix2[:], in0=io_t[:], in1=pen2[:], op=ALU.add)
                i2 = spool.tile([P, 1], F32, tag="rt_i2")
                nc.vector.tensor_reduce(i2[:], ix2[:], axis=AX.X, op=ALU.min)
                mask2 = spool.tile([P, E], F32, tag="rt_mask2")
                nc.vector.tensor_scalar(mask2[:], io_t[:], i2[:, 0:1], None,
                                        op0=ALU.is_equal)

                mask = spool.tile([P, E], F32, tag="rt_mask")
                nc.vector.tensor_tensor(out=mask[:], in0=mask1[:], in1=mask2[:], op=ALU.add)
                sm = spool.tile([P, E], F32, tag="rt_sm")
                nc.vector.tensor_tensor(out=sm[:], in0=St[:], in1=mask[:], op=ALU.mult)
                den = spool.tile([P, 1], F32, tag="rt_den")
                nc.vector.reduce_sum(den[:], sm[:], axis=AX.X)
                den2 = spool.tile([P, 1], F32, tag="rt_den2")
                nc.vector.tensor_scalar(den2[:], den[:], 1e-9, None, op0=ALU.add)
                rec = spool.tile([P, 1], F32, tag="rt_rec")
                nc.vector.reciprocal(rec[:], den2[:])
                rec4 = spool.tile([P, 1], F32, tag="rt_rec4")
                nc.vector.tensor_scalar(rec4[:], rec[:], 0.25, None, op0=ALU.mult)
                gates = spool.tile([P, E], F32, tag="rt_gates")
                nc.vector.tensor_scalar(gates[:], sm[:], rec4[:, 0:1], None, op0=ALU.mult)

                tr_ps = pp_s.tile([E, P], F32, space="PSUM", tag="ps_small")
                nc.tensor.transpose(out=tr_ps[:], in_=gates[:], identity=id_t[:])
                nc.vector.tensor_copy(gates_T[:, tt * P:(tt + 1) * P], tr_ps[:])

            # ---------------- experts ----------------
            h_t = hpool.tile([P, NKH_S * TOK], F32R)     # [p, kh*TOK + tok]
            y_t = ypool.tile([P, NKD * TOK], F32)        # [p, m2*TOK + tok]

            # --- shared experts (SwiGLU, w3 pre-scaled by 1/8 on host) ---
            for e in range(NS):
                w3blks = {}
                for m in range(NKH_S):
                    w1blk = wpool.tile([P, NKD * P], F32R, tag="w_in")
                    nc.sync.dma_start(
                        out=w1blk[:].rearrange("p (kd m) -> p kd m", m=P),
                        in_=sw1[e, :, m * P:(m + 1) * P].rearrange("(kd p) m -> p kd m", p=P),
                    )
                    w3blk = wpool.tile([P, NKD * P], F32R, tag="w_in3")
                    nc.sync.dma_start(
                        out=w3blk[:].rearrange("p (kd m) -> p kd m", m=P),
                        in_=sw3[e, :, m * P:(m + 1) * P].rearrange("(kd p) m -> p kd m", p=P),
                    )
                    for n in range(NN):
                        ph1 = pp_h1.tile([P, NV], F32, space="PSUM", tag="ph1")
                        ph3 = pp_h3.tile([P, NV], F32, space="PSUM", tag="ph3")
                        for kd in range(NKD):
                            xs = x_t[:, kd * TOK + n * NV: kd * TOK + n * NV + NV]
                            nc.tensor.matmul(ph1[:], w1blk[:, kd * P:(kd + 1) * P], xs,
                                             start=(kd == 0), stop=(kd == NKD - 1))
                        for kd in range(NKD):
                            xs = x_t[:, kd * TOK + n * NV: kd * TOK + n * NV + NV]
                            nc.tensor.matmul(ph3[:], w3blk[:, kd * P:(kd + 1) * P], xs,
                                             start=(kd == 0), stop=(kd == NKD - 1))
                        sil = spool.tile([P, NV], F32, tag="sil")
                        nc.scalar.activation(sil[:], ph1[:], AF.Silu)
                        hs = h_t[:, m * TOK + n * NV: m * TOK + n * NV + NV]
                        nc.vector.tensor_tensor(out=hs, in0=sil[:], in1=ph3[:], op=ALU.mult)

                for m2 in range(NKD):
                    w2blk = wpool.tile([P, NKH_S * P], F32R, tag="w_out")
                    nc.sync.dma_start(
                        out=w2blk[:].rearrange("p (kh m) -> p kh m", m=P),
                        in_=sw2[e, :, m2 * P:(m2 + 1) * P].rearrange("(kh p) m -> p kh m", p=P),
                    )
                    for n in range(NN):
                        py = pp_y.tile([P, NV], F32, space="PSUM", tag="py")
                        for kh in range(NKH_S):
                            hsl = h_t[:, kh * TOK + n * NV: kh * TOK + n * NV + NV]
                            nc.tensor.matmul(py[:], w2blk[:, kh * P:(kh + 1) * P], hsl,
                                             start=(kh == 0), stop=(kh == NKH_S - 1))
                        ysl = y_t[:, m2 * TOK + n * NV: m2 * TOK + n * NV + NV]
                        if e == 0:
                            nc.vector.tensor_copy(ysl, py[:])
                        else:
                            nc.vector.tensor_tensor(out=ysl, in0=ysl, in1=py[:], op=ALU.add)

            # --- routed experts (GELU MLP, dense with gate masking) ---
            for e in range(E):
                gb_row = spool.tile([1, TOK], F32R, tag="gbrow")
                nc.sync.dma_start(out=gb_row[:], in_=gates_T[e:e + 1, :])
                gb = spool.tile([P, TOK], F32, tag="gb")
                for n in range(NN):
                    gb_ps = pp_s.tile([P, NV], F32, space="PSUM", tag="ps_small")
                    nc.tensor.matmul(gb_ps[:], ones_r[0:1, :],
                                     gb_row[0:1, n * NV:(n + 1) * NV],
                                     start=True, stop=True)
                    nc.vector.tensor_copy(gb[:, n * NV:(n + 1) * NV], gb_ps[:])

                for m in range(NKH_R):
                    w1blk = wpool.tile([P, NKD * P], F32R, tag="w_in")
                    nc.sync.dma_start(
                        out=w1blk[:].rearrange("p (kd m) -> p kd m", m=P),
                        in_=rw1[e, :, m * P:(m + 1) * P].rearrange("(kd p) m -> p kd m", p=P),
                    )
                    for n in range(NN):
                        ph = pp_h1.tile([P, NV], F32, space="PSUM", tag="ph1")
                        for kd in range(NKD):
                            xs = x_t[:, kd * TOK + n * NV: kd * TOK + n * NV + NV]
                            nc.tensor.matmul(ph[:], w1blk[:, kd * P:(kd + 1) * P], xs,
                                             start=(kd == 0), stop=(kd == NKD - 1))
                        hg = spool.tile([P, NV], F32, tag="hg")
                        nc.scalar.activation(hg[:], ph[:], AF.Gelu)
                        hs = h_t[:, m * TOK + n * NV: m * TOK + n * NV + NV]
                        nc.vector.tensor_tensor(out=hs, in0=hg[:],
                                                in1=gb[:, n * NV:(n + 1) * NV], op=ALU.mult)

                for m2 in range(NKD):
                    w2blk = wpool.tile([P, NKH_S * P], F32R, tag="w_out")
                    nc.sync.dma_start(
                        out=w2blk[:, :NKH_R * P].rearrange("p (kh m) -> p kh m", m=P),
                        in_=rw2[e, :, m2 * P:(m2 + 1) * P].rearrange("(kh p) m -> p kh m", p=P),
                    )
                    for n in range(NN):
                        py = pp_y.tile([P, NV], F32, space="PSUM", tag="py")
                        for kh in range(NKH_R):
                            hsl = h_t[:, kh * TOK + n * NV: kh * TOK + n * NV + NV]
                            nc.tensor.matmul(py[:], w2blk[:, kh * P:(kh + 1) * P], hsl,
                                             start=(kh == 0), stop=(kh == NKH_R - 1))
                        ysl = y_t[:, m2 * TOK + n * NV: m2 * TOK + n * NV + NV]
                        nc.vector.tensor_tensor(out=ysl, in0=ysl, in1=py[:], op=ALU.add)

            # ---------------- store ----------------
            for m2 in range(NKD):
                nc.sync.dma_start(
                    out=yout[m2 * P:(m2 + 1) * P, :],
                    in_=y_t[:, m2 * TOK:(m2 + 1) * TOK],
                )

    _legalize_waits(nc)
    return nc


_CACHE = {}


def _prep_weights(t_emb, W_router, router_bias, s_w1, s_w3, s_w2, r_w1, r_w2):
    key = tuple(id(a) for a in (t_emb, W_router, router_bias, s_w1, s_w3, s_w2, r_w1, r_w2))
    hit = _CACHE.get("wkey")
    if hit is not None and hit[0] == key:
        return hit[1]
    assert np.all(np.asarray(router_bias) == 0.0), "kernel assumes zero router bias"
    c = np.ascontiguousarray
    f = np.float32
    prep = dict(
        wrT=c(np.asarray(W_router, f).T),                      # [2D, E] -> split cols
        sw1=c(np.asarray(s_w1, f).transpose(0, 2, 1)),          # [NS, D, HS]
        sw3=c(np.asarray(s_w3, f).transpose(0, 2, 1) / 8.0),    # scale folds /(NS*(NS+TOPK))
        sw2=c(np.asarray(s_w2, f).transpose(0, 2, 1)),          # [NS, HS, D]
        rw1=c(np.asarray(r_w1, f).transpose(0, 2, 1)),          # [E, D, HR]
        rw2=c(np.asarray(r_w2, f).transpose(0, 2, 1)),          # [E, HR, D]
        t_cols=[c(np.asarray(t_emb, f)[b].reshape(NKD, P).T) for b in range(B)],
        iota=c(np.broadcast_to(np.arange(E, dtype=f), (P, E))),
        ident=np.eye(P, dtype=f),
    )
    prep["wrT_packed"] = c(np.concatenate(
        [prep["wrT"][:D, :], prep["wrT"][D:, :]], axis=1))      # [D, 16]: x-part | t-part
    out = prep
    _CACHE["wkey"] = (key, out)
    return out


def kernel(x, t_emb, W_router, router_bias, s_w1, s_w3, s_w2, r_w1, r_w2):
    x = np.asarray(x, np.float32)
    pw = _prep_weights(t_emb, W_router, router_bias, s_w1, s_w3, s_w2, r_w1, r_w2)

    if "nc" not in _CACHE:
        _CACHE["nc"] = _build_nc()
    nc = _CACHE["nc"]

    xT_full = np.ascontiguousarray(x.reshape(B * T, D).T)       # [D, N]
    in_maps = []
    for cix in range(N_CORES):
        in_maps.append(dict(
            xT=np.ascontiguousarray(xT_full[:, cix * TOK:(cix + 1) * TOK]),
            tC=pw["t_cols"][cix * TOK // T],
            wrT=pw["wrT_packed"],
            iota=pw["iota"],
            ident=pw["ident"],
            sw1=pw["sw1"], sw3=pw["sw3"], sw2=pw["sw2"],
            rw1=pw["rw1"], rw2=pw["rw2"],
        ))

    res = run_bass_kernel_spmd(nc, in_maps, list(range(N_CORES)))

    out = np.empty((D, B * T), dtype=np.float32)
    for cix in range(N_CORES):
        out[:, cix * TOK:(cix + 1) * TOK] = res.results[cix]["yout"]
    return np.ascontiguousarray(out.T).reshape(B, T, D)
